# revision 12
# baseline (speedup 1.0000x reference)
"""GCN autoencoder (6x gcn_layer) on 8 TRN2 NeuronCores.

Strategy (v3):
  - Rows of adj_/X sharded across 8 cores; weights replicated.
  - All device tensors bf16 (fp32 PSUM accumulation); host does the free
    sharding / transposes / casts and the final gather+transpose.
  - adj-mm produces zT = (adj_shard @ H)^T so the next layer's XW matmul
    consumes it directly (no transposes anywhere on device).
  - Each layer runs three row-phases (256 / 512 / 256 local rows).  After
    phase p: XW(l+1) for those rows -> DRAM bounce -> AllGather.  The
    early-phase gathers land while the layer is still computing; the next
    layer consumes k-chunks in arrival order (wave0|wave1|wave2), so AG
    flight time stays off the PE critical path.
  - adj columns 0:256 and 768:1024 SBUF-resident; 256:768 streamed.
  - Layer 1's H1 = X @ W1 is computed fully on every core from the
    (replicated, free) input X -> no collective before the first adj-mm.
"""

import sys

import numpy as np

if "/opt/trn_rl_repo" not in sys.path:
    sys.path.insert(0, "/opt/trn_rl_repo")

import ml_dtypes

import concourse.bacc as bacc
import concourse.tile as tile
from concourse import mybir
from concourse.bass_utils import run_bass_kernel_spmd

N = 8192
D_IN = 512
NCORES = 8
R = N // NCORES  # 1024 rows per core
DIMS = [(512, 256), (256, 256), (256, 128), (128, 256), (256, 256), (256, 512)]

BF16 = mybir.dt.bfloat16
F32 = mybir.dt.float32
NP_BF16 = ml_dtypes.bfloat16
RELU = mybir.ActivationFunctionType.Relu

KO = N // 128  # 64 k-chunks over the gather dim
RT = R // 128  # 8 local row tiles
PHASES = [(0, 256), (256, 512), (768, 256)]  # (row offset, rows) per phase

_CACHED = {}


def _build():
    nc = bacc.Bacc(
        "TRN2",
        target_bir_lowering=False,
        debug=False,
        enable_asserts=False,
        num_devices=NCORES,
    )

    adjT = nc.dram_tensor("adjT", [N, R], BF16, kind="ExternalInput")
    xT = nc.dram_tensor("xT", [D_IN, N], BF16, kind="ExternalInput")
    w_dram = [
        nc.dram_tensor(f"W{i + 1}", list(DIMS[i]), BF16, kind="ExternalInput")
        for i in range(6)
    ]
    outT = nc.dram_tensor("outT", [DIMS[-1][1], R], F32, kind="ExternalOutput")

    adjT_r = adjT.ap().rearrange("(ko p) r -> p ko r", p=128)
    xT_r = xT.ap().rearrange("(kx p) c -> p kx c", p=128)

    with tile.TileContext(nc) as tc:
        with (
            tc.tile_pool(name="adjres", bufs=1) as adjres_p,
            tc.tile_pool(name="adjstr", bufs=3) as adjstr_p,
            tc.tile_pool(name="wp", bufs=1) as wp,
            tc.tile_pool(name="xtp", bufs=3) as xtp,
            tc.tile_pool(name="ztp", bufs=14) as ztp,
            tc.tile_pool(name="hp", bufs=3) as hp,
            tc.tile_pool(name="hstage", bufs=4) as hstage,
            tc.tile_pool(name="ostage", bufs=2) as ostage,
            tc.tile_pool(name="psz", bufs=5, space="PSUM") as psz,
            tc.tile_pool(name="psh", bufs=3, space="PSUM") as psh,
            tc.tile_pool(name="dram", bufs=1, space="DRAM") as dram,
        ):
            # ---- resident weights ----
            w_sb = []
            for i, (di, do) in enumerate(DIMS):
                w_t = wp.tile([128, di // 128, do], BF16, name=f"w{i}_sb")
                nc.sync.dma_start(
                    w_t[:], w_dram[i].ap().rearrange("(kx p) n -> p kx n", p=128)
                )
                w_sb.append(w_t)

            # ---- resident adj for the edge phases (cols 0:256, 768:1024);
            #      middle 256:768 streamed per layer.  DMAs for the resident
            #      parts are emitted after the layer-1 XW stream below.
            adj_res = {
                0: adjres_p.tile([128, KO, 256], BF16, name="adj_res0"),
                2: adjres_p.tile([128, KO, 256], BF16, name="adj_res2"),
            }
            adj_stream_cache = {}

            def adj_mov(g, pi):
                # stream in pairs of k-chunks: every consumption wave visits
                # complete g//2 pairs exactly once, so a pair's pool slot is
                # never needed again after its two chunks are consumed
                off, sz = PHASES[pi]
                if pi != 1:
                    return adj_res[pi][:, g, :]
                grp = g // 2
                t = adj_stream_cache.get(grp)
                if t is None:
                    t = adjstr_p.tile([128, 2, sz], BF16, tag="adjs",
                                      name=f"as{grp}")
                    nc.sync.dma_start(
                        t[:], adjT_r[:, grp * 2 : grp * 2 + 2, off : off + sz]
                    )
                    adj_stream_cache[grp] = t
                return t[:, g % 2, :]

            # ---- layer 1: H1 = X @ W1 computed fully on every core ----
            h_cur = hp.tile([128, KO, DIMS[0][1]], BF16, tag="h", name="h1")
            for g0 in range(0, KO, 2):
                xt_t = xtp.tile([128, D_IN // 128, 256], BF16, tag="xt")
                nc.sync.dma_start(xt_t[:], xT_r[:, :, g0 * 128 : g0 * 128 + 256])
                for g in (g0, g0 + 1):
                    ps_h = psh.tile([128, DIMS[0][1]], F32, tag="psh")
                    for kx in range(D_IN // 128):
                        c = (g - g0) * 128
                        nc.tensor.matmul(
                            ps_h[:],
                            xt_t[:, kx, c : c + 128],
                            w_sb[0][:, kx, :],
                            start=(kx == 0),
                            stop=(kx == D_IN // 128 - 1),
                        )
                    nc.vector.tensor_copy(h_cur[:, g, :], ps_h[:])

            # resident-adj loads, emitted after the XW1 stream so the small
            # xT/W DMAs get the queues first; k-ordered to match consumption
            for j in range(0, KO, 8):
                nc.sync.dma_start(
                    adj_res[0][:, j : j + 8, :], adjT_r[:, j : j + 8, 0:256]
                )
            for j in range(0, KO, 8):
                nc.sync.dma_start(
                    adj_res[2][:, j : j + 8, :], adjT_r[:, j : j + 8, 768:1024]
                )

            # k-chunk consumption order:
            #   layer 1: production order (g ascending);
            #   layers >=2: producer-phase arrival order
            #   (phase p delivers chunks {c*8 + off/128 + j, j < sz/128})
            k_order_l1 = list(range(KO))
            waves = [
                [c * RT + off // 128 + j
                 for c in range(NCORES) for j in range(sz // 128)]
                for (off, sz) in PHASES
            ]
            k_order_gather = [g for w in waves for g in w]

            for li, (di, do) in enumerate(DIMS):
                last = li == len(DIMS) - 1
                mt = do // 128
                korder = k_order_l1 if li == 0 else k_order_gather

                if not last:
                    di2, do2 = DIMS[li + 1]
                    kxn2 = di2 // 128  # == mt
                    if do2 <= 256:
                        h_next = [hp.tile([128, KO, do2], BF16, tag="h",
                                          name=f"h{li + 2}")]
                        nsplit = [(0, do2)]
                    else:  # layer 6: split H columns into two 256 buffers
                        h_next = [
                            hp.tile([128, KO, 256], BF16, tag="h",
                                    name=f"h{li + 2}a"),
                            hp.tile([128, KO, 256], BF16, tag="h",
                                    name=f"h{li + 2}b"),
                        ]
                        nsplit = [(0, 256), (256, 256)]

                def h_lhsT(m, g):
                    if isinstance(h_cur, list):
                        return h_cur[m // 2][:, g, (m % 2) * 128 : (m % 2) * 128 + 128]
                    return h_cur[:, g, m * 128 : (m + 1) * 128]

                for pi, (off, sz) in enumerate(PHASES):
                    # ---- adj-mm phase: zT[:, off:off+sz] ----
                    # k-outer so each streamed adj chunk is fetched once and
                    # shared by all m tiles; mt psum banks accumulate together.
                    adj_stream_cache.clear()
                    ps_zs = [psz.tile([128, sz], F32, tag="psz", name=f"psz{m}")
                             for m in range(mt)]
                    for ki, g in enumerate(korder):
                        mov = adj_mov(g, pi)
                        for m in range(mt):
                            nc.tensor.matmul(
                                ps_zs[m][:],
                                h_lhsT(m, g),
                                mov,
                                start=(ki == 0),
                                stop=(ki == KO - 1),
                            )
                    zt_p = []
                    for m in range(mt):
                        if last:
                            o_st = ostage.tile([128, sz], F32, tag="ost")
                            nc.scalar.activation(o_st[:], ps_zs[m][:], RELU)
                            nc.sync.dma_start(
                                outT[m * 128 : (m + 1) * 128, off : off + sz],
                                o_st[:],
                            )
                            zt_p.append(None)
                        else:
                            z_t = ztp.tile([128, sz], BF16, tag="zt",
                                           name=f"z{li + 1}_{m}_{pi}")
                            nc.scalar.activation(z_t[:], ps_zs[m][:], RELU)
                            zt_p.append(z_t)

                    if last:
                        continue

                    # ---- XW(l+1) for this phase's rows, then AllGather ----
                    bounces = [
                        dram.tile([sz, dc], BF16, tag=f"hb{li}_{pi}_{ci}",
                                  name=f"hb{li}_{pi}_{ci}")
                        for ci, (c0, dc) in enumerate(nsplit)
                    ]
                    for j in range(sz // 128):
                        ps_h = psh.tile([128, do2], F32, tag="psh")
                        for kx in range(kxn2):
                            nc.tensor.matmul(
                                ps_h[:],
                                zt_p[kx][:, j * 128 : (j + 1) * 128],
                                w_sb[li + 1][:, kx, :],
                                start=(kx == 0),
                                stop=(kx == kxn2 - 1),
                            )
                        for ci, (c0, dc) in enumerate(nsplit):
                            h_st = hstage.tile([128, dc], BF16, tag="hst")
                            nc.vector.tensor_copy(h_st[:], ps_h[:, c0 : c0 + dc])
                            nc.sync.dma_start(
                                bounces[ci][j * 128 : (j + 1) * 128, :], h_st[:]
                            )
                    nch = sz // 128  # chunks this phase contributes per core
                    for ci, (c0, dc) in enumerate(nsplit):
                        gath = dram.tile(
                            [NCORES * sz, dc], BF16, addr_space="Shared",
                            tag=f"hg{li}_{pi}_{ci}", name=f"hg{li}_{pi}_{ci}",
                        )
                        nc.gpsimd.collective_compute(
                            "AllGather",
                            mybir.AluOpType.bypass,
                            ins=[bounces[ci][:].opt()],
                            outs=[gath[:].opt()],
                            replica_groups=[list(range(NCORES))],
                        )
                        g_r = gath.rearrange("(q p) d -> p q d", p=128)
                        for c in range(NCORES):
                            nc.sync.dma_start(
                                h_next[ci][:, c * RT + off // 128 :
                                           c * RT + off // 128 + nch, :],
                                g_r[:, c * nch : (c + 1) * nch, :],
                            )

                if not last:
                    h_cur = h_next if len(h_next) > 1 else h_next[0]

    nc.compile()
    return nc


def kernel(**inputs):
    X = np.asarray(inputs["X"], dtype=np.float32)
    adj = np.asarray(inputs["adj_"], dtype=np.float32)

    if "nc" not in _CACHED:
        _CACHED["nc"] = _build()
    nc = _CACHED["nc"]

    xT_full = np.ascontiguousarray(X.T).astype(NP_BF16)
    ws = [np.asarray(inputs[f"W{j + 1}"], np.float32).astype(NP_BF16) for j in range(6)]
    in_maps = []
    for i in range(NCORES):
        rows = slice(i * R, (i + 1) * R)
        m = {
            "adjT": np.ascontiguousarray(adj[rows, :].T).astype(NP_BF16),
            "xT": xT_full,
        }
        for j in range(6):
            m[f"W{j + 1}"] = ws[j]
        in_maps.append(m)

    res = run_bass_kernel_spmd(nc, in_maps, core_ids=list(range(NCORES)))
    out = np.concatenate(
        [np.asarray(r["outT"], dtype=np.float32).T for r in res.results], axis=0
    )
    return out
